# revision 1
# baseline (speedup 1.0000x reference)
"""GRU-decoder kernel for 8 Trainium2 NeuronCores.

Math (all 127 output steps are identical — see the reference):
    x0   = relu(emb[input[:,0]])                       [B,H]
    h0   = einsum('blh,l->bh', hidden, bridge_w) + bb  [B,H]
    gi   = x0 @ w_ih.T + b_ih ; gh = h0 @ w_hh.T + b_hh
    r,z  = sigmoid(...) ; n = tanh(in + r*hn)
    h1   = (1-z)*n + z*h0
    logp = log_softmax(h1 @ proj_w.T + proj_b)         [B,V]
    out  = broadcast(logp, [B, L-1, V])

Sharding: vocab-parallel projection (each core owns V/8 rows of proj_w)
plus h-sharded GRU (each core owns a 128-wide slice of the hidden dim,
computes partial gate pre-activations, and one AllReduce combines them).
A small AllGather combines per-core softmax (max, sumexp) stats so the
global log-softmax normalizer is applied on device. The [B,V] result is
gathered on host and broadcast (a zero-copy view) over the L-1 steps.
"""

import numpy as np

import concourse.bass as bass
import concourse.tile as tile
from concourse import bacc, mybir
from concourse.bass_utils import run_bass_kernel_spmd

B, L, H, V = 16, 128, 1024, 50257
NC = 8
VC = 6656                # per-core vocab shard (13*512); 8*VC = 53248 >= V
HC = H // NC             # per-core hidden-dim shard (128)
G3 = 3 * H               # gate rows (r,z,n)
NT = G3 // 128           # 24 j-tiles of 128
NEG = -1.0e30

f32 = mybir.dt.float32
f32r = mybir.dt.float32r
FX = mybir.ActivationFunctionType
AX = mybir.AxisListType

# v-chunks of <=512 for PSUM; DMA groups of 4 chunks (2048 cols)
CHUNKS = [(i * 512, min(512, VC - i * 512)) for i in range((VC + 511) // 512)]
N_CH = len(CHUNKS)
GROUPS = [(g * 2048, min(2048, VC - g * 2048)) for g in range((VC + 2047) // 2048)]

LAST_RESULT = None  # test harness reads profiling info from here
_NC_CACHE = None


def _bc(ap, insert_at, step, count):
    """Insert a broadcast/strided dim into an AP at position insert_at."""
    new = list(ap.ap)
    new.insert(insert_at, [step, count])
    return bass.AP(tensor=ap.tensor, offset=ap.offset, ap=new)


def _build():
    nc = bacc.Bacc("TRN2", target_bir_lowering=False, debug=False, num_devices=NC)

    x0T = nc.dram_tensor("x0T", [HC, B], f32, kind="ExternalInput").ap()
    hid = nc.dram_tensor("hid", [B, L, HC], f32, kind="ExternalInput").ap()
    wihT = nc.dram_tensor("wihT", [HC, G3], f32, kind="ExternalInput").ap()
    whhT = nc.dram_tensor("whhT", [HC, G3], f32, kind="ExternalInput").ap()
    bih = nc.dram_tensor("bih", [G3], f32, kind="ExternalInput").ap()
    bhh = nc.dram_tensor("bhh", [G3], f32, kind="ExternalInput").ap()
    bw = nc.dram_tensor("bw", [L, 1], f32, kind="ExternalInput").ap()
    bb = nc.dram_tensor("bb", [1, 1], f32, kind="ExternalInput").ap()
    msk = nc.dram_tensor("msk", [1, NC], f32, kind="ExternalInput").ap()
    pwT = nc.dram_tensor("pwT", [H, VC], f32r, kind="ExternalInput").ap()
    pb = nc.dram_tensor("pb", [1, VC], f32, kind="ExternalInput").ap()
    logp = nc.dram_tensor("logp", [B, VC], f32, kind="ExternalOutput").ap()

    with tile.TileContext(nc) as tc:
        with (
            tc.tile_pool(name="singles", bufs=1) as singles,
            tc.tile_pool(name="gru_ps", bufs=1, space="PSUM") as gru_ps,
            tc.tile_pool(name="proj_ps", bufs=4, space="PSUM") as proj_ps,
            tc.tile_pool(name="pw", bufs=11) as pwpool,
            tc.tile_pool(name="stats", bufs=4) as stats,
            tc.tile_pool(name="dram", bufs=1, space="DRAM") as dram,
        ):
            # ---- small input loads ---------------------------------------
            x0T_sb = singles.tile([HC, B], f32, tag="x0T_sb")
            nc.sync.dma_start(out=x0T_sb, in_=x0T)
            nc.scalar.activation(out=x0T_sb[:], in_=x0T_sb[:], func=FX.Relu)

            hid_sb = singles.tile([L, B, HC], f32, tag="hid_sb")
            nc.sync.dma_start(out=hid_sb, in_=hid.rearrange("b l h -> l b h"))

            wih_sb = singles.tile([HC, G3], f32, tag="wih_sb")
            nc.sync.dma_start(out=wih_sb, in_=wihT)
            whh_sb = singles.tile([HC, G3], f32, tag="whh_sb")
            nc.sync.dma_start(out=whh_sb, in_=whhT)

            # biases in T layout: [128, 24] with partition = j%128, col = j//128
            biT = singles.tile([128, NT], f32, tag="biT")
            nc.sync.dma_start(out=biT, in_=bih.rearrange("(t p) -> p t", p=128))
            bhT = singles.tile([128, NT], f32, tag="bhT")
            nc.sync.dma_start(out=bhT, in_=bhh.rearrange("(t p) -> p t", p=128))
            bsum = singles.tile([128, 16], f32, tag="bsum")
            nc.vector.tensor_add(bsum, biT[:, 0:16], bhT[:, 0:16])

            bw_sb = singles.tile([L, 1], f32, tag="bw_sb")
            nc.sync.dma_start(out=bw_sb, in_=bw)
            bb_sb = singles.tile([128, 1], f32, tag="bb_sb")
            nc.sync.dma_start(out=bb_sb, in_=_bc(bb[0], 0, 0, 128))
            msk_sb = singles.tile([128, NC], f32, tag="msk_sb")
            nc.sync.dma_start(out=msk_sb, in_=_bc(msk[0], 0, 0, 128))

            pbb = singles.tile([B, VC], f32, tag="pbb")
            nc.sync.dma_start(out=pbb, in_=_bc(pb[0], 0, 0, B))

            # ---- bridge: h0T_c[h,b] = sum_l hidden[b,l,h]*w[l] -----------
            h0T_ps = gru_ps.tile([HC, B], f32, tag="h0T_ps")
            for b in range(B):
                nc.tensor.matmul(
                    h0T_ps[:, b : b + 1], hid_sb[:, b, :], bw_sb[:],
                    start=True, stop=True,
                )
            h0T_sb = singles.tile([HC, B], f32, tag="h0T_sb")
            nc.vector.tensor_scalar_add(h0T_sb[:], h0T_ps[:], bb_sb[:, 0:1])

            # ---- partial gate pre-activations (T layout) -----------------
            giT_ps = gru_ps.tile([128, NT, B], f32, tag="giT_ps")
            ghT_ps = gru_ps.tile([128, NT, B], f32, tag="ghT_ps")
            for t in range(NT):
                nc.tensor.matmul(
                    giT_ps[:, t, :], wih_sb[:, t * 128 : (t + 1) * 128], x0T_sb[:],
                    start=True, stop=True,
                )
                nc.tensor.matmul(
                    ghT_ps[:, t, :], whh_sb[:, t * 128 : (t + 1) * 128], h0T_sb[:],
                    start=True, stop=True,
                )

            # ---- pack AllReduce payload [128, 56, 16] --------------------
            arbuf = singles.tile([128, 2 * NT + NC, B], f32, tag="arbuf")
            nc.vector.tensor_copy(arbuf[:, 0:NT, :], giT_ps[:])
            nc.vector.tensor_copy(arbuf[:, NT : 2 * NT, :], ghT_ps[:])
            h0_bcast = _bc(h0T_sb[:], 1, 0, NC)          # [128, 8, 16]
            msk_bcast = _bc(msk_sb[:], 2, 0, B)          # [128, 8, 16]
            nc.vector.tensor_mul(arbuf[:, 2 * NT :, :], h0_bcast, msk_bcast)

            cc_in = dram.tile([128, (2 * NT + NC) * B], f32, tag="cc_in")
            cc_out = dram.tile([128, (2 * NT + NC) * B], f32, tag="cc_out")
            nc.sync.dma_start(out=cc_in[:], in_=arbuf[:])
            nc.gpsimd.collective_compute(
                "AllReduce",
                mybir.AluOpType.add,
                replica_groups=[list(range(NC))],
                ins=[cc_in.opt()],
                outs=[cc_out.opt()],
            )
            arx = singles.tile([128, 2 * NT + NC, B], f32, tag="arx")
            nc.sync.dma_start(out=arx[:], in_=cc_out[:])

            # ---- gates (full width, every core redundantly) --------------
            rT = singles.tile([128, NC, B], f32, tag="rT")
            nc.vector.tensor_add(rT[:], arx[:, 0:8, :], arx[:, 24:32, :])
            nc.vector.tensor_add(rT[:], rT[:], _bc(bsum[:, 0:8], 2, 0, B))
            nc.scalar.activation(out=rT[:], in_=rT[:], func=FX.Sigmoid)

            zT = singles.tile([128, NC, B], f32, tag="zT")
            nc.vector.tensor_add(zT[:], arx[:, 8:16, :], arx[:, 32:40, :])
            nc.vector.tensor_add(zT[:], zT[:], _bc(bsum[:, 8:16], 2, 0, B))
            nc.scalar.activation(out=zT[:], in_=zT[:], func=FX.Sigmoid)

            nT = singles.tile([128, NC, B], f32, tag="nT")
            nc.vector.tensor_add(nT[:], arx[:, 40:48, :], _bc(bhT[:, 16:24], 2, 0, B))
            nc.vector.tensor_mul(nT[:], nT[:], rT[:])
            nc.vector.tensor_add(nT[:], nT[:], arx[:, 16:24, :])
            nc.vector.tensor_add(nT[:], nT[:], _bc(biT[:, 16:24], 2, 0, B))
            nc.scalar.activation(out=nT[:], in_=nT[:], func=FX.Tanh)

            h1T = singles.tile([128, NC, B], f32, tag="h1T")
            nc.vector.tensor_mul(h1T[:], zT[:], arx[:, 48:56, :])   # z*h0
            nc.vector.tensor_mul(zT[:], zT[:], nT[:])               # z*n
            nc.vector.tensor_add(h1T[:], h1T[:], nT[:])             # + n
            nc.vector.tensor_sub(h1T[:], h1T[:], zT[:])             # - z*n
            h1Tr = singles.tile([128, NC, B], f32r, tag="h1Tr")
            nc.vector.tensor_copy(h1Tr[:], h1T[:])

            # ---- projection + online softmax -----------------------------
            logits_sb = singles.tile([B, VC], f32, tag="logits_sb")
            m_run = singles.tile([B, 1], f32, tag="m_run")
            s_run = singles.tile([B, 1], f32, tag="s_run")
            nc.vector.memset(m_run, -1.0e38)
            nc.vector.memset(s_run, 0.0)

            pw_view = pwT.rearrange("(kc p) v -> kc p v", p=128)
            for gi_, (gcol, gw) in enumerate(GROUPS):
                gtiles = []
                for kc in range(NC):
                    t = pwpool.tile([128, 2048], f32r, tag="pwt")
                    nc.sync.dma_start(
                        out=t[:, :gw], in_=pw_view[kc, :, gcol : gcol + gw]
                    )
                    gtiles.append(t)
                for sub in range((gw + 511) // 512):
                    col = gcol + sub * 512
                    nv = min(512, VC - col)
                    lg = proj_ps.tile([B, 512], f32, tag="lg")
                    for kc in range(NC):
                        nc.tensor.matmul(
                            lg[:, :nv],
                            h1Tr[:, kc, :],
                            gtiles[kc][:, sub * 512 : sub * 512 + nv],
                            start=(kc == 0), stop=(kc == NC - 1),
                        )
                    nc.vector.tensor_add(
                        logits_sb[:, col : col + nv], lg[:, :nv],
                        pbb[:, col : col + nv],
                    )

                    cmax = stats.tile([B, 1], f32, tag="cmax")
                    nc.vector.reduce_max(cmax, logits_sb[:, col : col + nv], axis=AX.X)
                    new_m = stats.tile([B, 1], f32, tag="new_m")
                    nc.vector.tensor_max(new_m, m_run, cmax)
                    neg_m = stats.tile([B, 1], f32, tag="neg_m")
                    nc.vector.tensor_scalar_mul(neg_m, new_m, -1.0)
                    scale = stats.tile([B, 1], f32, tag="scale")
                    nc.scalar.activation(
                        out=scale, in_=m_run, func=FX.Exp, bias=neg_m[:, 0:1]
                    )
                    expb = stats.tile([B, 512], f32, tag="expb")
                    csum = stats.tile([B, 1], f32, tag="csum")
                    nc.scalar.activation(
                        out=expb[:, :nv], in_=logits_sb[:, col : col + nv], func=FX.Exp,
                        bias=neg_m[:, 0:1], accum_out=csum[:, 0:1],
                    )
                    nc.vector.tensor_mul(s_run, s_run, scale)
                    nc.vector.tensor_add(s_run, s_run, csum)
                    nc.vector.tensor_copy(m_run, new_m)

            # ---- global softmax stats (AllGather) ------------------------
            std_in = dram.tile([2, B], f32, tag="std_in")
            std_out = dram.tile([NC * 2, B], f32, tag="std_out")
            nc.sync.dma_start(out=std_in[0:1, :], in_=m_run[:])
            nc.sync.dma_start(out=std_in[1:2, :], in_=s_run[:])
            nc.gpsimd.collective_compute(
                "AllGather",
                mybir.AluOpType.bypass,
                replica_groups=[list(range(NC))],
                ins=[std_in.opt()],
                outs=[std_out.opt()],
            )
            mstats = singles.tile([B, NC, 2], f32, tag="mstats")
            so = std_out[:]  # [16, B] dram AP, row = 2c+j
            nc.sync.dma_start(
                out=mstats,
                in_=bass.AP(
                    tensor=so.tensor, offset=so.offset,
                    ap=[[1, B], [2 * B, NC], [B, 2]],
                ),
            )
            gM = singles.tile([B, 1], f32, tag="gM")
            nc.vector.reduce_max(gM, mstats[:, :, 0], axis=AX.X)
            ngM = singles.tile([B, 1], f32, tag="ngM")
            nc.vector.tensor_scalar_mul(ngM, gM, -1.0)
            em = singles.tile([B, NC], f32, tag="em")
            nc.scalar.activation(
                out=em, in_=mstats[:, :, 0], func=FX.Exp, bias=ngM[:, 0:1]
            )
            nc.vector.tensor_mul(em, em, mstats[:, :, 1])
            gS = singles.tile([B, 1], f32, tag="gS")
            nc.vector.reduce_sum(gS, em, axis=AX.X)
            nc.scalar.activation(out=gS, in_=gS, func=FX.Ln)
            nc.vector.tensor_add(gM, gM, gS)               # lse
            nc.vector.tensor_scalar_mul(gM, gM, -1.0)      # -lse

            # ---- logp = logits - lse, write out --------------------------
            nc.vector.tensor_scalar_add(logits_sb[:], logits_sb[:], gM[:, 0:1])
            nc.sync.dma_start(out=logp, in_=logits_sb[:])

    nc.compile()
    return nc


def kernel(input, hidden, emb, bridge_w, bridge_b, w_ih, w_hh, b_ih, b_hh,
           proj_w, proj_b):
    global _NC_CACHE, LAST_RESULT
    if _NC_CACHE is None:
        _NC_CACHE = _build()
    nc = _NC_CACHE

    input = np.asarray(input)
    hidden = np.asarray(hidden, dtype=np.float32)
    emb = np.asarray(emb, dtype=np.float32)
    bridge_w = np.asarray(bridge_w, dtype=np.float32)
    bridge_b = np.asarray(bridge_b, dtype=np.float32)
    w_ih = np.asarray(w_ih, dtype=np.float32)
    w_hh = np.asarray(w_hh, dtype=np.float32)
    b_ih = np.asarray(b_ih, dtype=np.float32)
    b_hh = np.asarray(b_hh, dtype=np.float32)
    proj_w = np.asarray(proj_w, dtype=np.float32)
    proj_b = np.asarray(proj_b, dtype=np.float32)

    x0 = emb[input[:, 0].astype(np.int64)]          # [B, H]
    x0T = np.ascontiguousarray(x0.T)                # [H, B]
    bw_in = np.ascontiguousarray(bridge_w.reshape(L, 1))
    bb_in = bridge_b.reshape(1, 1)

    in_maps = []
    for c in range(NC):
        hs = slice(c * HC, (c + 1) * HC)
        lo, hi = c * VC, min((c + 1) * VC, V)
        pw_blk = proj_w[lo:hi]
        pb_blk = proj_b[lo:hi]
        if hi - lo < VC:
            pad = VC - (hi - lo)
            pw_blk = np.concatenate([pw_blk, np.zeros((pad, H), np.float32)], axis=0)
            pb_blk = np.concatenate([pb_blk, np.full((pad,), NEG, np.float32)])
        onehot = np.zeros((1, NC), np.float32)
        onehot[0, c] = 1.0
        in_maps.append({
            "x0T": np.ascontiguousarray(x0T[hs]),
            "hid": np.ascontiguousarray(hidden[:, :, hs]),
            "wihT": np.ascontiguousarray(w_ih[:, hs].T),
            "whhT": np.ascontiguousarray(w_hh[:, hs].T),
            "bih": b_ih,
            "bhh": b_hh,
            "bw": bw_in,
            "bb": bb_in,
            "msk": onehot,
            "pwT": np.ascontiguousarray(pw_blk.T),
            "pb": np.ascontiguousarray(pb_blk.reshape(1, VC)),
        })

    res = run_bass_kernel_spmd(nc, in_maps, list(range(NC)))
    LAST_RESULT = res

    logp_full = np.concatenate([res.results[c]["logp"] for c in range(NC)], axis=1)
    logp_full = np.ascontiguousarray(logp_full[:, :V])
    return np.broadcast_to(logp_full[:, None, :], (B, L - 1, V))



# revision 3
# speedup vs baseline: 1.4491x; 1.4491x over previous
"""GRU-decoder kernel for 8 Trainium2 NeuronCores (v2).

Math (all 127 output steps are identical -- see the reference):
    x0   = relu(emb[input[:,0]])                       [B,H]
    h0   = einsum('blh,l->bh', hidden, bridge_w) + bb  [B,H]
    gi   = x0 @ w_ih.T + b_ih ; gh = h0 @ w_hh.T + b_hh
    r,z  = sigmoid(...) ; n = tanh(in + r*hn)
    h1   = (1-z)*n + z*h0
    logp = log_softmax(h1 @ proj_w.T + proj_b)         [B,V]
    out  = broadcast(logp, [B, L-1, V])

Sharding: vocab-parallel projection (each core owns VC=6400 rows of
proj_w) plus h-sharded GRU (each core owns a 128-wide slice of the
hidden dim, computes partial gate pre-activations, one slim bf16
AllReduce combines them). Projection weights are fp8e4 (scaled x2048 on
host, folded back via activation scale) and use the DoubleRow perf mode
(2 fp8 MACs/cell/cycle, K=256 per pass). All projection weights are
prefetched into SBUF during the GRU phase so the PE stream is never
DMA-starved. Softmax needs no max subtraction (logits are O(1) by
construction), so per-chunk stats are a single fused exp+accumulate;
one tiny AllGather combines per-core sum-exp for the global
normalizer.
"""

import numpy as np
import ml_dtypes

import concourse.bass as bass
import concourse.tile as tile
from concourse import bacc, mybir
from concourse.bass_utils import run_bass_kernel_spmd

B, L, H, V = 16, 128, 1024, 50257
NC = 8
HC = H // NC             # per-core hidden-dim shard (128)
G3 = 3 * H               # gate rows (r,z,n)
NT = G3 // 128           # 24 j-tiles of 128
VC = 6400                # per-core vocab shard; 8*VC = 51200 >= V
KD = 4                   # double-K chunks (4 x 256 = 1024) for fp8 DoubleRow
NEG = -1.0e30
SCL = 2048.0             # host scales proj_w by this; device folds 1/SCL into exp/copy
SINV = 1.0 / SCL

f32 = mybir.dt.float32
bf16 = mybir.dt.bfloat16
f8 = mybir.dt.float8e4
FX = mybir.ActivationFunctionType
AX = mybir.AxisListType
ALU = mybir.AluOpType
PM = mybir.MatmulPerfMode
F8NP = ml_dtypes.float8_e4m3

CHUNKS = [(i * 512, min(512, VC - i * 512)) for i in range((VC + 511) // 512)]

LAST_RESULT = None  # test harness reads profiling info from here
_NC_CACHE = None


def _bc(ap, insert_at, step, count):
    """Insert a broadcast/strided dim into an AP at position insert_at."""
    new = list(ap.ap)
    new.insert(insert_at, [step, count])
    return bass.AP(tensor=ap.tensor, offset=ap.offset, ap=new)


def _build():
    nc = bacc.Bacc("TRN2", target_bir_lowering=False, debug=False, num_devices=NC)

    x0T = nc.dram_tensor("x0T", [HC, B], f32, kind="ExternalInput").ap()
    hidT = nc.dram_tensor("hidT", [L, B, HC], bf16, kind="ExternalInput").ap()
    bw = nc.dram_tensor("bw", [L, 1], bf16, kind="ExternalInput").ap()
    bb = nc.dram_tensor("bb", [1, 1], f32, kind="ExternalInput").ap()
    wihT = nc.dram_tensor("wihT", [HC, G3], bf16, kind="ExternalInput").ap()
    whhT = nc.dram_tensor("whhT", [HC, G3], bf16, kind="ExternalInput").ap()
    brz = nc.dram_tensor("brz", [128, 16], f32, kind="ExternalInput").ap()
    bin_ = nc.dram_tensor("bin", [128, 8], f32, kind="ExternalInput").ap()
    bhn = nc.dram_tensor("bhn", [128, 8], f32, kind="ExternalInput").ap()
    msk = nc.dram_tensor("msk", [1, NC], f32, kind="ExternalInput").ap()
    pwq = nc.dram_tensor("pwq", [KD * 128, 2 * VC], f8, kind="ExternalInput").ap()
    pb2 = nc.dram_tensor("pb2", [1, VC], f32, kind="ExternalInput").ap()
    logp = nc.dram_tensor("logp", [B, VC], f32, kind="ExternalOutput").ap()

    pw_view = pwq.rearrange("(d p) v -> d p v", p=128)

    with tile.TileContext(nc) as tc:
        with (
            tc.tile_pool(name="singles", bufs=1) as singles,
            tc.tile_pool(name="gru_ps", bufs=1, space="PSUM") as gru_ps,
            tc.tile_pool(name="proj_ps", bufs=4, space="PSUM") as proj_ps,
            tc.tile_pool(name="stats", bufs=4) as stats,
            tc.tile_pool(name="dram", bufs=1, space="DRAM") as dram,
        ):
            # ---- small input loads (issued first: they gate the GRU) -----
            x0T_sb = singles.tile([HC, B], f32, tag="x0T_sb")
            nc.sync.dma_start(out=x0T_sb, in_=x0T)
            bw_sb = singles.tile([L, 1], bf16, tag="bw_sb")
            nc.sync.dma_start(out=bw_sb, in_=bw)
            bb_sb = singles.tile([128, 1], f32, tag="bb_sb")
            nc.sync.dma_start(out=bb_sb, in_=_bc(bb[0], 0, 0, 128))
            brz_sb = singles.tile([128, 16], f32, tag="brz_sb")
            nc.sync.dma_start(out=brz_sb, in_=brz)
            bin_sb = singles.tile([128, 8], f32, tag="bin_sb")
            nc.sync.dma_start(out=bin_sb, in_=bin_)
            bhn_sb = singles.tile([128, 8], f32, tag="bhn_sb")
            nc.sync.dma_start(out=bhn_sb, in_=bhn)
            msk_sb = singles.tile([128, NC], f32, tag="msk_sb")
            nc.sync.dma_start(out=msk_sb, in_=_bc(msk[0], 0, 0, 128))

            hid_sb = singles.tile([L, B, HC], bf16, tag="hid_sb")
            nc.sync.dma_start(out=hid_sb, in_=hidT)
            wih_sb = singles.tile([HC, G3], bf16, tag="wih_sb")
            nc.sync.dma_start(out=wih_sb, in_=wihT)
            whh_sb = singles.tile([HC, G3], bf16, tag="whh_sb")
            nc.sync.dma_start(out=whh_sb, in_=whhT)

            # ---- projection weights: all resident, prefetched ------------
            pwt = []
            for d in range(KD):
                t = singles.tile([128, 2, VC], f8, tag=f"pw{d}")
                nc.sync.dma_start(out=t[:], in_=pw_view[d])
                pwt.append(t)
            pbb = singles.tile([B, VC], f32, tag="pbb")
            nc.sync.dma_start(out=pbb, in_=_bc(pb2[0], 0, 0, B))

            # ---- x0 relu + bf16 cast -------------------------------------
            nc.scalar.activation(out=x0T_sb[:], in_=x0T_sb[:], func=FX.Relu)
            x0bf = singles.tile([HC, B], bf16, tag="x0bf")
            nc.vector.tensor_copy(x0bf[:], x0T_sb[:])

            # ---- bridge: h0T[h,b] = sum_l hid[l,b,h]*w[l] ----------------
            h0T_ps = gru_ps.tile([HC, B], f32, tag="h0T_ps")
            for b in range(B):
                nc.tensor.matmul(
                    h0T_ps[:, b : b + 1], hid_sb[:, b, :], bw_sb[:],
                    start=True, stop=True,
                )
            h0T_sb = singles.tile([HC, B], f32, tag="h0T_sb")
            nc.vector.tensor_scalar_add(h0T_sb[:], h0T_ps[:], bb_sb[:, 0:1])
            h0bf = singles.tile([HC, B], bf16, tag="h0bf")
            nc.vector.tensor_copy(h0bf[:], h0T_sb[:])

            # ---- partial gate pre-activations (T layout) -----------------
            # r,z rows (t<16): PE accumulates gi+gh directly in PSUM.
            giT_ps = gru_ps.tile([128, NT, B], f32, tag="giT_ps")
            ghT_ps = gru_ps.tile([128, 8, B], f32, tag="ghT_ps")
            for t in range(NT):
                if t < 16:
                    nc.tensor.matmul(
                        giT_ps[:, t, :], wih_sb[:, t * 128 : (t + 1) * 128], x0bf[:],
                        start=True, stop=False,
                    )
                    nc.tensor.matmul(
                        giT_ps[:, t, :], whh_sb[:, t * 128 : (t + 1) * 128], h0bf[:],
                        start=False, stop=True,
                    )
                else:
                    nc.tensor.matmul(
                        giT_ps[:, t, :], wih_sb[:, t * 128 : (t + 1) * 128], x0bf[:],
                        start=True, stop=True,
                    )
                    nc.tensor.matmul(
                        ghT_ps[:, t - 16, :], whh_sb[:, t * 128 : (t + 1) * 128], h0bf[:],
                        start=True, stop=True,
                    )

            # ---- pack slim AllReduce payload [128, 40, 16] bf16 ----------
            # slots 0:16  = gi+gh for r,z rows   (pre-added in PSUM)
            # slots 16:24 = gi for n rows (in)
            # slots 24:32 = gh for n rows (hn)
            # slots 32:40 = h0 shard (masked -> allgather-by-sum)
            arbuf = singles.tile([128, 40, B], bf16, tag="arbuf")
            nc.vector.tensor_copy(arbuf[:, 0:24, :], giT_ps[:])
            nc.vector.tensor_copy(arbuf[:, 24:32, :], ghT_ps[:])
            h0_bcast = _bc(h0T_sb[:], 1, 0, NC)          # [128, 8, 16]
            msk_bcast = _bc(msk_sb[:], 2, 0, B)          # [128, 8, 16]
            nc.vector.tensor_mul(arbuf[:, 32:40, :], h0_bcast, msk_bcast)

            cc_in = dram.tile([128, 40 * B], bf16, tag="cc_in")
            cc_out = dram.tile([128, 40 * B], bf16, tag="cc_out", addr_space="Shared")
            nc.sync.dma_start(out=cc_in[:], in_=arbuf[:])
            nc.gpsimd.collective_compute(
                "AllReduce",
                ALU.add,
                replica_groups=[list(range(NC))],
                ins=[cc_in.opt()],
                outs=[cc_out.opt()],
            )
            arx = singles.tile([128, 40, B], bf16, tag="arx")
            nc.sync.dma_start(out=arx[:], in_=cc_out[:])

            # ---- gates (full width, every core redundantly) --------------
            rT = singles.tile([128, 8, B], f32, tag="rT")
            nc.vector.tensor_add(rT[:], arx[:, 0:8, :], _bc(brz_sb[:, 0:8], 2, 0, B))
            nc.scalar.activation(out=rT[:], in_=rT[:], func=FX.Sigmoid)

            zT = singles.tile([128, 8, B], f32, tag="zT")
            nc.vector.tensor_add(zT[:], arx[:, 8:16, :], _bc(brz_sb[:, 8:16], 2, 0, B))
            nc.scalar.activation(out=zT[:], in_=zT[:], func=FX.Sigmoid)

            nT = singles.tile([128, 8, B], f32, tag="nT")
            nc.vector.tensor_add(nT[:], arx[:, 24:32, :], _bc(bhn_sb[:], 2, 0, B))
            nc.vector.tensor_mul(nT[:], nT[:], rT[:])
            nc.vector.tensor_add(nT[:], nT[:], arx[:, 16:24, :])
            nc.vector.tensor_add(nT[:], nT[:], _bc(bin_sb[:], 2, 0, B))
            nc.scalar.activation(out=nT[:], in_=nT[:], func=FX.Tanh)

            h1T = singles.tile([128, 8, B], f32, tag="h1T")
            nc.vector.tensor_mul(h1T[:], zT[:], arx[:, 32:40, :])   # z*h0
            nc.vector.tensor_mul(zT[:], zT[:], nT[:])               # z*n
            nc.vector.tensor_add(h1T[:], h1T[:], nT[:])             # + n
            nc.vector.tensor_sub(h1T[:], h1T[:], zT[:])             # - z*n
            h1f8 = singles.tile([128, 8, B], f8, tag="h1f8")
            nc.vector.tensor_copy(h1f8[:], h1T[:])

            # ---- projection (fp8 DoubleRow) + online sum-exp -------------
            logits_sb = singles.tile([B, VC], f32, tag="logits_sb")
            s_run = singles.tile([B, 1], f32, tag="s_run")
            nc.vector.memset(s_run, 0.0)

            for ci, (col, nv) in enumerate(CHUNKS):
                lg = proj_ps.tile([B, 512], f32, tag="lg")
                for d in range(KD):
                    nc.tensor.matmul(
                        lg[:, :nv],
                        h1f8[:, 2 * d : 2 * d + 2, :],
                        pwt[d][:, :, col : col + nv],
                        start=(d == 0), stop=(d == KD - 1),
                        perf_mode=PM.DoubleRow,
                    )
                nc.vector.tensor_add(
                    logits_sb[:, col : col + nv], lg[:, :nv], pbb[:, col : col + nv]
                )
                expb = stats.tile([B, 512], f32, tag="expb")
                csum = stats.tile([B, 1], f32, tag="csum")
                nc.scalar.activation(
                    out=expb[:, :nv], in_=logits_sb[:, col : col + nv], func=FX.Exp,
                    scale=SINV, accum_out=csum[:, 0:1],
                )
                nc.vector.tensor_add(s_run, s_run, csum)

            # ---- global sum-exp (AllGather) + lse ------------------------
            std_in = dram.tile([1, B], f32, tag="std_in")
            std_out = dram.tile([NC, B], f32, tag="std_out", addr_space="Shared")
            nc.sync.dma_start(out=std_in[0:1, :], in_=s_run[:])
            nc.gpsimd.collective_compute(
                "AllGather",
                ALU.bypass,
                replica_groups=[list(range(NC))],
                ins=[std_in.opt()],
                outs=[std_out.opt()],
            )
            sg = singles.tile([B, NC], f32, tag="sg")
            so = std_out[:]
            nc.sync.dma_start(
                out=sg,
                in_=bass.AP(
                    tensor=so.tensor, offset=so.offset,
                    ap=[[1, B], [B, NC]],
                ),
            )
            gS = singles.tile([B, 1], f32, tag="gS")
            nc.vector.reduce_sum(gS, sg, axis=AX.X)
            nc.scalar.activation(out=gS, in_=gS, func=FX.Ln)
            nc.vector.tensor_scalar_mul(gS, gS, -1.0)      # -lse (of true logits)

            # ---- logp = logits*SINV - lse; DVE and ACT split the pass ----
            HALF = 3072
            nc.vector.tensor_scalar(
                out=logits_sb[:, :HALF], in0=logits_sb[:, :HALF],
                scalar1=SINV, scalar2=gS[:, 0:1],
                op0=ALU.mult, op1=ALU.add,
            )
            nc.scalar.activation(
                out=logits_sb[:, HALF:], in_=logits_sb[:, HALF:],
                func=FX.Identity, scale=SINV, bias=gS[:, 0:1],
            )
            nc.sync.dma_start(out=logp, in_=logits_sb[:])

    nc.compile()
    return nc


def kernel(input, hidden, emb, bridge_w, bridge_b, w_ih, w_hh, b_ih, b_hh,
           proj_w, proj_b):
    global _NC_CACHE, LAST_RESULT
    if _NC_CACHE is None:
        _NC_CACHE = _build()
    nc = _NC_CACHE

    input = np.asarray(input)
    hidden = np.asarray(hidden, dtype=np.float32)
    emb = np.asarray(emb, dtype=np.float32)
    bridge_w = np.asarray(bridge_w, dtype=np.float32)
    bridge_b = np.asarray(bridge_b, dtype=np.float32)
    w_ih = np.asarray(w_ih, dtype=np.float32)
    w_hh = np.asarray(w_hh, dtype=np.float32)
    b_ih = np.asarray(b_ih, dtype=np.float32)
    b_hh = np.asarray(b_hh, dtype=np.float32)
    proj_w = np.asarray(proj_w, dtype=np.float32)
    proj_b = np.asarray(proj_b, dtype=np.float32)

    x0 = emb[input[:, 0].astype(np.int64)]          # [B, H]
    x0T = np.ascontiguousarray(x0.T)                # [H, B]
    hidT = hidden.transpose(1, 0, 2)                # [L, B, H]
    bw_in = bridge_w.reshape(L, 1).astype(ml_dtypes.bfloat16)
    bb_in = bridge_b.reshape(1, 1)

    bsum = b_ih + b_hh
    brz_in = np.ascontiguousarray(bsum[: 2 * H].reshape(16, 128).T)
    bin_in = np.ascontiguousarray(b_ih[2 * H :].reshape(8, 128).T)
    bhn_in = np.ascontiguousarray(b_hh[2 * H :].reshape(8, 128).T)

    in_maps = []
    for c in range(NC):
        hs = slice(c * HC, (c + 1) * HC)
        lo, hi = c * VC, min((c + 1) * VC, V)
        pw_blk = proj_w[lo:hi]
        pb_blk = proj_b[lo:hi]
        if hi - lo < VC:
            pad = VC - (hi - lo)
            pw_blk = np.concatenate([pw_blk, np.zeros((pad, H), np.float32)], axis=0)
            pb_blk = np.concatenate([pb_blk, np.full((pad,), NEG, np.float32)])
        # fp8 DoubleRow layout: [(d p), (ko v)] with k = d*256 + ko*128 + p
        pw8 = np.clip(pw_blk.T * SCL, -240.0, 240.0).astype(F8NP)   # [H, VC]
        pwq_in = np.ascontiguousarray(
            pw8.reshape(KD, 2, 128, VC).transpose(0, 2, 1, 3)
        ).reshape(KD * 128, 2 * VC)
        onehot = np.zeros((1, NC), np.float32)
        onehot[0, c] = 1.0
        in_maps.append({
            "x0T": np.ascontiguousarray(x0T[hs]),
            "hidT": np.ascontiguousarray(hidT[:, :, hs]).astype(ml_dtypes.bfloat16),
            "bw": bw_in,
            "bb": bb_in,
            "wihT": np.ascontiguousarray(w_ih[:, hs].T).astype(ml_dtypes.bfloat16),
            "whhT": np.ascontiguousarray(w_hh[:, hs].T).astype(ml_dtypes.bfloat16),
            "brz": brz_in,
            "bin": bin_in,
            "bhn": bhn_in,
            "msk": onehot,
            "pwq": pwq_in,
            "pb2": np.ascontiguousarray((pb_blk * SCL).reshape(1, VC)),
        })

    res = run_bass_kernel_spmd(nc, in_maps, list(range(NC)))
    LAST_RESULT = res

    logp_full = np.concatenate([res.results[c]["logp"] for c in range(NC)], axis=1)
    logp_full = np.ascontiguousarray(logp_full[:, :V])
    return np.broadcast_to(logp_full[:, None, :], (B, L - 1, V))
